# revision 10
# baseline (speedup 1.0000x reference)
"""Trainium2 Bass kernel for the binary-conv BasicBlock (dense_cnn).

Computation (forward values only):
  A1   = sign(x + b11)
  out1 = x + bn1(conv3x3(A1, binw(w3)))          binw(w) = mean|w| * sign(w)
  o1   = prelu(out1 + b12, a1) + b13
  A2   = sign(o1 + b21)
  out2 = bn2(conv1x1(A2, binw(w1))) + o1
  out  = prelu(out2 + b22, a2) + b23

Strategy: data-parallel over the batch axis, 4 images per NeuronCore on 8
cores; weights/consts replicated.  Per core the 3x3 binary conv runs as 9
shifted 256x256 matmuls (bf16 +-1 operands, fp32 PSUM accumulation) over a
zero-padded 58x58 activation layout, the 1x1 conv as a plain channel matmul.
All per-channel affine factors are folded:
  - weight scale * BN slope folded into the bf16 weights
  - BN intercept + b12 folded into x on the host (x_prep = x + K1)
  - remaining per-channel adds ride activation bias / tensor_scalar slots
  - prelu(t, a) = max(a*t, t)   (valid for a <= 1; numpy fallback otherwise)
"""

import numpy as np
import ml_dtypes

C = 256
H = W = 56
PH = 58                    # padded image side
NPIX = PH * PH             # 3364
HALO = 59                  # extra zero halo so all 9 shift-reads stay in range
ACT1W = HALO + NPIX + HALO # 3482
A1BLK = 3488               # act1 per-K-half block (16B aligned for DoubleRow)
A2BLK = 3376               # act2 per-K-half block
BPC = 4                    # images per core
NCORES = 8
EPS = 1e-5
NTILES = [(t0, min(512, NPIX - t0)) for t0 in range(0, NPIX, 512)]

_CACHE = {}


def _split_drain_waits(m, max_waits=1):
    """This toolchain's walrus rejects instructions carrying more than ~1-2
    sync waits; hoist extra waits onto preceding single-wait EventSemaphore
    ops on the same engine (semantically identical: the engine blocks on
    each wait in sequence before executing the instruction)."""
    import copy as _copy
    from concourse import mybir

    new_module = _copy.replace(m, functions=[])
    for function in m.functions:
        new_function = _copy.replace(function, blocks=[])
        new_function.set_allocations_from_list(function.allocations)
        for block in function.blocks:
            out = []
            for inst in block.instructions:
                si = inst.sync_info
                if si is not None and len(si.on_wait) > max_waits:
                    waits = list(si.on_wait)
                    keep = waits[:max_waits] if not isinstance(
                        inst, mybir.InstDrain) else []
                    hoist = waits[len(keep):]
                    for i, wt in enumerate(hoist):
                        out.append(
                            mybir.InstEventSemaphore(
                                name=f"{inst.name}-wsplit{i}",
                                opcode="EventSemaphore",
                                engine=inst.engine,
                                sync_info=mybir.SyncInfo(on_wait=[wt], on_update=[]),
                            )
                        )
                    inst.sync_info = mybir.SyncInfo(
                        on_wait=keep, on_update=list(si.on_update)
                    )
                out.append(inst)
            new_block = _copy.replace(block, instructions=out)
            new_function.blocks.append(new_block)
        new_module.functions.append(new_function)
    return new_module


def build_nc():
    """Build (once) the per-core Bass program."""
    if "nc" in _CACHE:
        return _CACHE["nc"]
    import concourse.bass as bass
    import concourse.tile as tile
    from concourse import mybir

    Alu = mybir.AluOpType
    AF = mybir.ActivationFunctionType
    f32 = mybir.dt.float32
    bf16 = mybir.dt.bfloat16

    nc = bass.Bass(trn_type="TRN2", debug=False)
    x_d = nc.dram_tensor("xprep", [BPC, 2, 128, NPIX], f32, kind="ExternalInput")
    fp8 = mybir.dt.float8e4
    DR = mybir.MatmulPerfMode.DoubleRow
    w3_d = nc.dram_tensor("w3f", [128, 9 * 2 * 2 * 128], fp8, kind="ExternalInput")
    w1_d = nc.dram_tensor("w1f", [128, 2 * 2 * 128], fp8, kind="ExternalInput")
    c_d = nc.dram_tensor("consts", [2, 128, 8], f32, kind="ExternalInput")
    o_d = nc.dram_tensor("out", [BPC, 2, 128, H * W], f32, kind="ExternalOutput")

    def interior(ap_2d, width):
        # [128, width] AP over padded pixels -> [128, 56, 56] interior view
        return ap_2d.rearrange("p (h w) -> p h w", h=PH)[:, 1:57, 1:57]

    with tile.TileContext(nc) as tc:
        with (
            tc.tile_pool(name="wpool", bufs=1) as wpool,
            tc.tile_pool(name="xpool", bufs=2) as xpool,
            tc.tile_pool(name="apool", bufs=2) as apool,
            tc.tile_pool(name="ppool", bufs=1) as ppool,
            tc.tile_pool(name="tpool", bufs=1) as tpool,
            tc.tile_pool(name="opool", bufs=2) as opool,
            tc.tile_pool(name="ps1", bufs=5, space="PSUM") as ps1p,
            tc.tile_pool(name="ps2", bufs=3, space="PSUM") as ps2p,
        ):
            # ---- constants / weights (resident) ----
            w3sb = wpool.tile([128, 9 * 2 * 2 * 128], fp8, tag="w3")
            nc.sync.dma_start(w3sb[:], w3_d.ap())
            w1sb = wpool.tile([128, 2 * 2 * 128], fp8, tag="w1")
            nc.sync.dma_start(w1sb[:], w1_d.ap())
            # [p, (sh mc), 2, m] / [p, mc, 2, m] views for DoubleRow lhsT
            w3v = w3sb[:].rearrange("p (g two m) -> p g two m", two=2, m=128)
            w1v = w1sb[:].rearrange("p (g two m) -> p g two m", two=2, m=128)
            csb = []
            for kc in range(2):
                ct = wpool.tile([128, 8], f32, tag=f"c_{kc}")
                nc.sync.dma_start(ct[:], c_d.ap()[kc])
                csb.append(ct)

            def cc(kc, j):
                return csb[kc][:, j : j + 1]

            # per-image state (rotating through pool slots)
            xts = [None] * BPC   # x_prep tiles per chunk
            a1ts = [None] * BPC  # act1 (bf16, halo layout)
            a2ts = [None] * BPC  # act2 (bf16, padded layout)
            p1ts = [None] * BPC  # p1 (prelu output, padded layout)

            def prep(img):
                xc = []
                at = apool.tile([128, 2 * A1BLK], fp8, tag="act1", name="a1")
                for kc in range(2):
                    xt = xpool.tile([128, NPIX], f32, tag=f"x_{kc}")
                    # x_prep arrives host-padded (zero border): contiguous DMA
                    nc.sync.dma_start(xt[:], x_d.ap()[img, kc])

                    b = kc * A1BLK
                    nc.gpsimd.memset(at[:, b : b + 118], 0.0)
                    bv = at[:, b + 174 : b + 174 + 56 * 58].rearrange(
                        "p (h w) -> p h w", h=56
                    )[:, :, 0:2]
                    nc.gpsimd.memset(bv, 0.0)
                    nc.gpsimd.memset(at[:, b + 3366 : b + A1BLK], 0.0)
                    # A1 = sign(x_prep + (b11 - K1)), written interior-only
                    nc.scalar.activation(
                        interior(at[:, b + HALO : b + HALO + NPIX], NPIX),
                        interior(xt[:], NPIX),
                        AF.Sign,
                        bias=cc(kc, 0),
                    )
                    xc.append(xt)
                xts[img] = xc
                a1ts[img] = at

            def conv1(img, prep_next=None):
                a2t = apool.tile([128, 2 * A2BLK], fp8, tag="act2", name="a2")
                p1c = [
                    ppool.tile([128, NPIX], f32, tag=f"p1_{kc}", name=f"p1_{kc}") for kc in range(2)
                ]
                a1v = a1ts[img][:].rearrange("p (two w) -> p two w", two=2)
                for ti, (t0, n) in enumerate(NTILES):
                    if ti == 4 and prep_next is not None:
                        prep(prep_next)
                    for mc in range(2):
                        ps = ps1p.tile([128, 512], f32, tag="ps1")
                        for sh in range(9):
                            kh, kw = divmod(sh, 3)
                            off = HALO + t0 + (kh - 1) * PH + (kw - 1)
                            nc.tensor.matmul(
                                ps[:, :n],
                                w3v[:, sh * 2 + mc],
                                a1v[:, :, off : off + n],
                                start=(sh == 0),
                                stop=(sh == 8),
                                perf_mode=DR,
                            )
                        p1s = p1c[mc][:, t0 : t0 + n]
                        # t = psum*sh1 + x_prep  (= x + bn1 + b12, all folded)
                        nc.vector.scalar_tensor_tensor(
                            p1s, ps[:, :n], cc(mc, 6),
                            xts[img][mc][:, t0 : t0 + n], Alu.mult, Alu.add
                        )
                        # p1 = max(a1*t, t) = prelu(t, a1)
                        nc.vector.scalar_tensor_tensor(
                            p1s, p1s, cc(mc, 3), p1s, Alu.mult, Alu.max
                        )
                        # A2 = sign(p1 + (b13 + b21))
                        nc.scalar.activation(
                            a2t[:, mc * A2BLK + t0 : mc * A2BLK + t0 + n],
                            p1s, AF.Sign, bias=cc(mc, 1)
                        )
                a2ts[img] = a2t
                p1ts[img] = p1c

            def conv2(img):
                t2c = [
                    tpool.tile([128, NPIX], f32, tag=f"t2_{mc}", name=f"t2_{mc}") for mc in range(2)
                ]
                a2v = a2ts[img][:].rearrange("p (two w) -> p two w", two=2)
                for mc in range(2):
                    for t0, n in NTILES:
                        ps = ps2p.tile([128, 512], f32, tag="ps2")
                        nc.tensor.matmul(
                            ps[:, :n],
                            w1v[:, mc],
                            a2v[:, :, t0 : t0 + n],
                            start=True,
                            stop=True,
                            perf_mode=DR,
                        )
                        t2s = t2c[mc][:, t0 : t0 + n]
                        # t2 = (psum*sh2 + K2) + p1
                        nc.scalar.activation(
                            t2s, ps[:, :n], AF.Identity,
                            bias=cc(mc, 2), scale=cc(mc, 7)
                        )
                        nc.vector.tensor_tensor(
                            t2s, t2s, p1ts[img][mc][:, t0 : t0 + n], Alu.add
                        )
                        # out2 = max(a2*t2, t2) = prelu(t2, a2)
                        nc.vector.scalar_tensor_tensor(
                            t2s, t2s, cc(mc, 4), t2s, Alu.mult, Alu.max
                        )
                    # out = out2 + b23, compacted to 56x56 for a contiguous DMA
                    oc = opool.tile([128, H * W], f32, tag=f"o_{mc}", name=f"o_{mc}")
                    nc.scalar.activation(
                        oc[:].rearrange("p (h w) -> p h w", h=H),
                        interior(t2c[mc][:], NPIX),
                        AF.Identity,
                        bias=cc(mc, 5),
                    )
                    nc.sync.dma_start(o_d.ap()[img, mc], oc[:])

            prep(0)
            for img in range(BPC):
                conv1(img, prep_next=img + 1 if img + 1 < BPC else None)
                conv2(img)

    _CACHE["nc"] = nc
    return nc


def _host_fold(w3, w1, b11, b12, b13, b21, b22, b23,
               g1, be1, m1, v1, g2, be2, m2, v2, a1, a2):
    f = np.float32
    s3 = np.mean(np.abs(w3), axis=(1, 2, 3)).astype(f)
    s1 = np.mean(np.abs(w1), axis=(1, 2, 3)).astype(f)
    inv1 = (g1 / np.sqrt(v1 + EPS)).astype(f)
    inv2 = (g2 / np.sqrt(v2 + EPS)).astype(f)
    sh1 = s3 * inv1
    ch1 = be1 - m1 * inv1
    sh2 = s1 * inv2
    ch2 = be2 - m2 * inv2
    K1 = (ch1 + b12).astype(f)
    K2 = (ch2 + b13 + b22).astype(f)
    bias1 = (b11 - K1).astype(f)
    bias2 = (b13 + b21).astype(f)

    fp8 = ml_dtypes.float8_e4m3
    # DoubleRow lhsT layout: [k, ((sh*2+mc)*2+i)*128+m] with i the K-half
    W3 = np.sign(w3).astype(fp8)                                # [O, I, 3, 3]
    W3 = W3.reshape(2, 128, 2, 128, 3, 3)                       # [mc, m, i, k, kh, kw]
    W3 = W3.transpose(3, 4, 5, 0, 2, 1)                         # [k, kh, kw, mc, i, m]
    W3f = np.ascontiguousarray(W3.reshape(128, 9 * 2 * 2 * 128))
    W1 = np.sign(w1).astype(fp8)                                # [O, I, 1, 1]
    W1 = W1.reshape(2, 128, 2, 128)                             # [mc, m, i, k]
    W1 = W1.transpose(3, 0, 2, 1)                               # [k, mc, i, m]
    W1f = np.ascontiguousarray(W1.reshape(128, 2 * 2 * 128))

    consts = np.zeros((2, 128, 8), f)
    for kc in range(2):
        sl = slice(kc * 128, (kc + 1) * 128)
        consts[kc, :, 0] = bias1[sl]
        consts[kc, :, 1] = bias2[sl]
        consts[kc, :, 2] = K2[sl]
        consts[kc, :, 3] = a1[sl]
        consts[kc, :, 4] = a2[sl]
        consts[kc, :, 5] = b23[sl]
        consts[kc, :, 6] = sh1[sl]
        consts[kc, :, 7] = sh2[sl]
    return W3f, W1f, consts, K1


def _run(in_maps, trace=False, tmpdir=None, trace_kwargs={}):
    from concourse import bass_utils

    nc = build_nc()
    if not _CACHE.get("split"):
        # walrus workaround applied only for the HW path (CoreSim rejects
        # post-scheduling instruction edits)
        nc.m = _split_drain_waits(nc.m)
        _CACHE["split"] = True
    return bass_utils.run_bass_kernel_spmd(
        nc,
        in_maps,
        core_ids=list(range(NCORES)),
        trace=trace,
        tmpdir=tmpdir,
        trace_kwargs=trace_kwargs,
    )


def make_in_maps(x, w3, w1, **params):
    x = np.asarray(x, np.float32)
    W3f, W1f, consts, K1 = _host_fold(np.asarray(w3, np.float32),
                                      np.asarray(w1, np.float32),
                                      **{k: np.asarray(v, np.float32)
                                         for k, v in params.items()})
    xp = np.zeros((x.shape[0], C, PH, PH), np.float32)
    xp[:, :, 1:57, 1:57] = x + K1[None, :, None, None]
    x_prep = xp.reshape(NCORES, BPC, 2, 128, NPIX)
    return [
        {"xprep": np.ascontiguousarray(x_prep[c]), "w3f": W3f, "w1f": W1f,
         "consts": consts}
        for c in range(NCORES)
    ]


def assemble_out(results):
    outs = [results[c]["out"].reshape(BPC, C, H, W) for c in range(NCORES)]
    return np.ascontiguousarray(
        np.concatenate(outs, axis=0).astype(np.float32)
    )


def _fallback_numpy(x, w3, w1, b11, b12, b13, b21, b22, b23,
                    g1, be1, m1, v1, g2, be2, m2, v2, a1, a2):
    # Straightforward reference math in numpy; only used if an assumption of
    # the device kernel (prelu slope <= 1) is violated.
    def cb(p):
        return p[None, :, None, None]

    def conv_np(a, w, pad):
        N, Ci, Hh, Ww = a.shape
        O, I, kh, kw = w.shape
        ap = np.pad(a, ((0, 0), (0, 0), (pad, pad), (pad, pad)))
        out = np.zeros((N, O, Hh, Ww), np.float32)
        wm = w.reshape(O, -1)
        for n in range(N):
            cols = np.empty((I * kh * kw, Hh * Ww), np.float32)
            idx = 0
            for i in range(I):
                for dh in range(kh):
                    for dw in range(kw):
                        cols[idx] = ap[n, i, dh : dh + Hh, dw : dw + Ww].ravel()
                        idx += 1
            out[n] = (wm @ cols).reshape(O, Hh, Ww)
        return out

    def bn(t, g, b, mm, v):
        inv = g / np.sqrt(v + EPS)
        return t * cb(inv) + cb(b - mm * inv)

    def prelu(t, a):
        return np.where(t > 0, t, cb(a) * t)

    s3 = np.mean(np.abs(w3), axis=(1, 2, 3), keepdims=True)
    s1 = np.mean(np.abs(w1), axis=(1, 2, 3), keepdims=True)
    o1 = conv_np(np.sign(x + cb(b11)), np.sign(w3) * s3, 1)
    o1 = x + bn(o1, g1, be1, m1, v1)
    o1 = prelu(o1 + cb(b12), a1) + cb(b13)
    o2 = conv_np(np.sign(o1 + cb(b21)), np.sign(w1) * s1, 0)
    o2 = bn(o2, g2, be2, m2, v2) + o1
    o2 = prelu(o2 + cb(b22), a2) + cb(b23)
    return o2.astype(np.float32)


def kernel(**inputs):
    inputs = {k: np.asarray(v) for k, v in inputs.items()}
    if (np.asarray(inputs["a1"]) > 1).any() or (np.asarray(inputs["a2"]) > 1).any():
        return _fallback_numpy(**{k: np.asarray(v, np.float32)
                                  for k, v in inputs.items()})
    in_maps = make_in_maps(**inputs)
    res = _run(in_maps, trace=False)
    return assemble_out(res.results)


# revision 11
# speedup vs baseline: 1.0227x; 1.0227x over previous
"""Trainium2 Bass kernel for the binary-conv BasicBlock (dense_cnn).

Computation (forward values only):
  A1   = sign(x + b11)
  out1 = x + bn1(conv3x3(A1, binw(w3)))          binw(w) = mean|w| * sign(w)
  o1   = prelu(out1 + b12, a1) + b13
  A2   = sign(o1 + b21)
  out2 = bn2(conv1x1(A2, binw(w1))) + o1
  out  = prelu(out2 + b22, a2) + b23

Strategy: data-parallel over the batch axis, 4 images per NeuronCore on 8
cores; weights/consts replicated.  Per core the 3x3 binary conv runs as 9
shifted 256x256 matmuls (bf16 +-1 operands, fp32 PSUM accumulation) over a
zero-padded 58x58 activation layout, the 1x1 conv as a plain channel matmul.
All per-channel affine factors are folded:
  - weight scale * BN slope folded into the bf16 weights
  - BN intercept + b12 folded into x on the host (x_prep = x + K1)
  - remaining per-channel adds ride activation bias / tensor_scalar slots
  - prelu(t, a) = max(a*t, t)   (valid for a <= 1; numpy fallback otherwise)
"""

import numpy as np
import ml_dtypes

C = 256
H = W = 56
PH = 58                    # padded image side
NPIX = PH * PH             # 3364
HALO = 59                  # extra zero halo so all 9 shift-reads stay in range
ACT1W = HALO + NPIX + HALO # 3482
A1BLK = 3488               # act1 per-K-half block (16B aligned for DoubleRow)
A2BLK = 3376               # act2 per-K-half block
BPC = 4                    # images per core
NCORES = 8
EPS = 1e-5
NTILES = [(t0, min(512, NPIX - t0)) for t0 in range(0, NPIX, 512)]

_CACHE = {}


def _split_drain_waits(m, max_waits=1):
    """This toolchain's walrus rejects instructions carrying more than ~1-2
    sync waits; hoist extra waits onto preceding single-wait EventSemaphore
    ops on the same engine (semantically identical: the engine blocks on
    each wait in sequence before executing the instruction)."""
    import copy as _copy
    from concourse import mybir

    new_module = _copy.replace(m, functions=[])
    for function in m.functions:
        new_function = _copy.replace(function, blocks=[])
        new_function.set_allocations_from_list(function.allocations)
        for block in function.blocks:
            out = []
            for inst in block.instructions:
                si = inst.sync_info
                if si is not None and len(si.on_wait) > max_waits:
                    waits = list(si.on_wait)
                    keep = waits[:max_waits] if not isinstance(
                        inst, mybir.InstDrain) else []
                    hoist = waits[len(keep):]
                    for i, wt in enumerate(hoist):
                        out.append(
                            mybir.InstEventSemaphore(
                                name=f"{inst.name}-wsplit{i}",
                                opcode="EventSemaphore",
                                engine=inst.engine,
                                sync_info=mybir.SyncInfo(on_wait=[wt], on_update=[]),
                            )
                        )
                    inst.sync_info = mybir.SyncInfo(
                        on_wait=keep, on_update=list(si.on_update)
                    )
                out.append(inst)
            new_block = _copy.replace(block, instructions=out)
            new_function.blocks.append(new_block)
        new_module.functions.append(new_function)
    return new_module


def build_nc():
    """Build (once) the per-core Bass program."""
    if "nc" in _CACHE:
        return _CACHE["nc"]
    import concourse.bass as bass
    import concourse.tile as tile
    from concourse import mybir

    Alu = mybir.AluOpType
    AF = mybir.ActivationFunctionType
    f32 = mybir.dt.float32
    bf16 = mybir.dt.bfloat16

    nc = bass.Bass(trn_type="TRN2", debug=False)
    x_d = nc.dram_tensor("xprep", [BPC, 2, 128, NPIX], f32, kind="ExternalInput")
    fp8 = mybir.dt.float8e4
    DR = mybir.MatmulPerfMode.DoubleRow
    w3_d = nc.dram_tensor("w3f", [128, 9 * 2 * 2 * 128], fp8, kind="ExternalInput")
    w1_d = nc.dram_tensor("w1f", [128, 2 * 2 * 128], fp8, kind="ExternalInput")
    c_d = nc.dram_tensor("consts", [2, 128, 8], f32, kind="ExternalInput")
    o_d = nc.dram_tensor("out", [BPC, 2, 128, H * W], f32, kind="ExternalOutput")

    def interior(ap_2d, width):
        # [128, width] AP over padded pixels -> [128, 56, 56] interior view
        return ap_2d.rearrange("p (h w) -> p h w", h=PH)[:, 1:57, 1:57]

    with tile.TileContext(nc) as tc:
        with (
            tc.tile_pool(name="wpool", bufs=1) as wpool,
            tc.tile_pool(name="xpool", bufs=2) as xpool,
            tc.tile_pool(name="apool", bufs=2) as apool,
            tc.tile_pool(name="ppool", bufs=1) as ppool,
            tc.tile_pool(name="tpool", bufs=1) as tpool,
            tc.tile_pool(name="opool", bufs=2) as opool,
            tc.tile_pool(name="ps1", bufs=6, space="PSUM") as ps1p,
            tc.tile_pool(name="ps2", bufs=2, space="PSUM") as ps2p,
        ):
            # ---- constants / weights (resident) ----
            w3sb = wpool.tile([128, 9 * 2 * 2 * 128], fp8, tag="w3")
            nc.sync.dma_start(w3sb[:], w3_d.ap())
            w1sb = wpool.tile([128, 2 * 2 * 128], fp8, tag="w1")
            nc.sync.dma_start(w1sb[:], w1_d.ap())
            # [p, (sh mc), 2, m] / [p, mc, 2, m] views for DoubleRow lhsT
            w3v = w3sb[:].rearrange("p (g two m) -> p g two m", two=2, m=128)
            w1v = w1sb[:].rearrange("p (g two m) -> p g two m", two=2, m=128)
            csb = []
            for kc in range(2):
                ct = wpool.tile([128, 8], f32, tag=f"c_{kc}")
                nc.sync.dma_start(ct[:], c_d.ap()[kc])
                csb.append(ct)

            def cc(kc, j):
                return csb[kc][:, j : j + 1]

            # per-image state (rotating through pool slots)
            xts = [None] * BPC   # x_prep tiles per chunk
            a1ts = [None] * BPC  # act1 (bf16, halo layout)
            a2ts = [None] * BPC  # act2 (bf16, padded layout)
            p1ts = [None] * BPC  # p1 (prelu output, padded layout)

            def prep(img):
                xc = []
                at = apool.tile([128, 2 * A1BLK], fp8, tag="act1", name="a1")
                for kc in range(2):
                    xt = xpool.tile([128, NPIX], f32, tag=f"x_{kc}")
                    # x_prep arrives host-padded (zero border): contiguous DMA
                    nc.sync.dma_start(xt[:], x_d.ap()[img, kc])

                    b = kc * A1BLK
                    nc.gpsimd.memset(at[:, b : b + 118], 0.0)
                    bv = at[:, b + 174 : b + 174 + 56 * 58].rearrange(
                        "p (h w) -> p h w", h=56
                    )[:, :, 0:2]
                    nc.gpsimd.memset(bv, 0.0)
                    nc.gpsimd.memset(at[:, b + 3366 : b + A1BLK], 0.0)
                    # A1 = sign(x_prep + (b11 - K1)), written interior-only
                    nc.scalar.activation(
                        interior(at[:, b + HALO : b + HALO + NPIX], NPIX),
                        interior(xt[:], NPIX),
                        AF.Sign,
                        bias=cc(kc, 0),
                    )
                    xc.append(xt)
                xts[img] = xc
                a1ts[img] = at

            def conv1(img, prep_next=None):
                a2t = apool.tile([128, 2 * A2BLK], fp8, tag="act2", name="a2")
                p1c = [
                    ppool.tile([128, NPIX], f32, tag=f"p1_{kc}", name=f"p1_{kc}") for kc in range(2)
                ]
                a1v = a1ts[img][:].rearrange("p (two w) -> p two w", two=2)
                for ti, (t0, n) in enumerate(NTILES):
                    if ti == 4 and prep_next is not None:
                        prep(prep_next)
                    for mc in range(2):
                        ps = ps1p.tile([128, 512], f32, tag="ps1")
                        for sh in range(9):
                            kh, kw = divmod(sh, 3)
                            off = HALO + t0 + (kh - 1) * PH + (kw - 1)
                            nc.tensor.matmul(
                                ps[:, :n],
                                w3v[:, sh * 2 + mc],
                                a1v[:, :, off : off + n],
                                start=(sh == 0),
                                stop=(sh == 8),
                                perf_mode=DR,
                            )
                        p1s = p1c[mc][:, t0 : t0 + n]
                        # t = psum*sh1 + x_prep  (= x + bn1 + b12, all folded)
                        nc.vector.scalar_tensor_tensor(
                            p1s, ps[:, :n], cc(mc, 6),
                            xts[img][mc][:, t0 : t0 + n], Alu.mult, Alu.add
                        )
                        # p1 = max(a1*t, t) = prelu(t, a1)
                        nc.vector.scalar_tensor_tensor(
                            p1s, p1s, cc(mc, 3), p1s, Alu.mult, Alu.max
                        )
                        # A2 = sign(p1 + (b13 + b21))
                        nc.scalar.activation(
                            a2t[:, mc * A2BLK + t0 : mc * A2BLK + t0 + n],
                            p1s, AF.Sign, bias=cc(mc, 1)
                        )
                a2ts[img] = a2t
                p1ts[img] = p1c

            def conv2(img):
                t2c = [
                    tpool.tile([128, NPIX], f32, tag=f"t2_{mc}", name=f"t2_{mc}") for mc in range(2)
                ]
                a2v = a2ts[img][:].rearrange("p (two w) -> p two w", two=2)
                for t0, n in NTILES:
                    for mc in range(2):
                        ps = ps2p.tile([128, 512], f32, tag="ps2")
                        nc.tensor.matmul(
                            ps[:, :n],
                            w1v[:, mc],
                            a2v[:, :, t0 : t0 + n],
                            start=True,
                            stop=True,
                            perf_mode=DR,
                        )
                        t2s = t2c[mc][:, t0 : t0 + n]
                        # t2 = (psum*sh2 + K2) + p1
                        nc.scalar.activation(
                            t2s, ps[:, :n], AF.Identity,
                            bias=cc(mc, 2), scale=cc(mc, 7)
                        )
                        nc.vector.tensor_tensor(
                            t2s, t2s, p1ts[img][mc][:, t0 : t0 + n], Alu.add
                        )
                        # out2 = max(a2*t2, t2) = prelu(t2, a2)
                        nc.vector.scalar_tensor_tensor(
                            t2s, t2s, cc(mc, 4), t2s, Alu.mult, Alu.max
                        )
                for mc in range(2):
                    # out = out2 + b23, compacted to 56x56 for a contiguous DMA
                    oc = opool.tile([128, H * W], f32, tag=f"o_{mc}", name=f"o_{mc}")
                    nc.scalar.activation(
                        oc[:].rearrange("p (h w) -> p h w", h=H),
                        interior(t2c[mc][:], NPIX),
                        AF.Identity,
                        bias=cc(mc, 5),
                    )
                    nc.sync.dma_start(o_d.ap()[img, mc], oc[:])

            prep(0)
            for img in range(BPC):
                conv1(img, prep_next=img + 1 if img + 1 < BPC else None)
                conv2(img)

    _CACHE["nc"] = nc
    return nc


def _host_fold(w3, w1, b11, b12, b13, b21, b22, b23,
               g1, be1, m1, v1, g2, be2, m2, v2, a1, a2):
    f = np.float32
    s3 = np.mean(np.abs(w3), axis=(1, 2, 3)).astype(f)
    s1 = np.mean(np.abs(w1), axis=(1, 2, 3)).astype(f)
    inv1 = (g1 / np.sqrt(v1 + EPS)).astype(f)
    inv2 = (g2 / np.sqrt(v2 + EPS)).astype(f)
    sh1 = s3 * inv1
    ch1 = be1 - m1 * inv1
    sh2 = s1 * inv2
    ch2 = be2 - m2 * inv2
    K1 = (ch1 + b12).astype(f)
    K2 = (ch2 + b13 + b22).astype(f)
    bias1 = (b11 - K1).astype(f)
    bias2 = (b13 + b21).astype(f)

    fp8 = ml_dtypes.float8_e4m3
    # DoubleRow lhsT layout: [k, ((sh*2+mc)*2+i)*128+m] with i the K-half
    W3 = np.sign(w3).astype(fp8)                                # [O, I, 3, 3]
    W3 = W3.reshape(2, 128, 2, 128, 3, 3)                       # [mc, m, i, k, kh, kw]
    W3 = W3.transpose(3, 4, 5, 0, 2, 1)                         # [k, kh, kw, mc, i, m]
    W3f = np.ascontiguousarray(W3.reshape(128, 9 * 2 * 2 * 128))
    W1 = np.sign(w1).astype(fp8)                                # [O, I, 1, 1]
    W1 = W1.reshape(2, 128, 2, 128)                             # [mc, m, i, k]
    W1 = W1.transpose(3, 0, 2, 1)                               # [k, mc, i, m]
    W1f = np.ascontiguousarray(W1.reshape(128, 2 * 2 * 128))

    consts = np.zeros((2, 128, 8), f)
    for kc in range(2):
        sl = slice(kc * 128, (kc + 1) * 128)
        consts[kc, :, 0] = bias1[sl]
        consts[kc, :, 1] = bias2[sl]
        consts[kc, :, 2] = K2[sl]
        consts[kc, :, 3] = a1[sl]
        consts[kc, :, 4] = a2[sl]
        consts[kc, :, 5] = b23[sl]
        consts[kc, :, 6] = sh1[sl]
        consts[kc, :, 7] = sh2[sl]
    return W3f, W1f, consts, K1


def _run(in_maps, trace=False, tmpdir=None, trace_kwargs={}):
    from concourse import bass_utils

    nc = build_nc()
    if not _CACHE.get("split"):
        # walrus workaround applied only for the HW path (CoreSim rejects
        # post-scheduling instruction edits)
        nc.m = _split_drain_waits(nc.m)
        _CACHE["split"] = True
    return bass_utils.run_bass_kernel_spmd(
        nc,
        in_maps,
        core_ids=list(range(NCORES)),
        trace=trace,
        tmpdir=tmpdir,
        trace_kwargs=trace_kwargs,
    )


def make_in_maps(x, w3, w1, **params):
    x = np.asarray(x, np.float32)
    W3f, W1f, consts, K1 = _host_fold(np.asarray(w3, np.float32),
                                      np.asarray(w1, np.float32),
                                      **{k: np.asarray(v, np.float32)
                                         for k, v in params.items()})
    xp = np.zeros((x.shape[0], C, PH, PH), np.float32)
    xp[:, :, 1:57, 1:57] = x + K1[None, :, None, None]
    x_prep = xp.reshape(NCORES, BPC, 2, 128, NPIX)
    return [
        {"xprep": np.ascontiguousarray(x_prep[c]), "w3f": W3f, "w1f": W1f,
         "consts": consts}
        for c in range(NCORES)
    ]


def assemble_out(results):
    outs = [results[c]["out"].reshape(BPC, C, H, W) for c in range(NCORES)]
    return np.ascontiguousarray(
        np.concatenate(outs, axis=0).astype(np.float32)
    )


def _fallback_numpy(x, w3, w1, b11, b12, b13, b21, b22, b23,
                    g1, be1, m1, v1, g2, be2, m2, v2, a1, a2):
    # Straightforward reference math in numpy; only used if an assumption of
    # the device kernel (prelu slope <= 1) is violated.
    def cb(p):
        return p[None, :, None, None]

    def conv_np(a, w, pad):
        N, Ci, Hh, Ww = a.shape
        O, I, kh, kw = w.shape
        ap = np.pad(a, ((0, 0), (0, 0), (pad, pad), (pad, pad)))
        out = np.zeros((N, O, Hh, Ww), np.float32)
        wm = w.reshape(O, -1)
        for n in range(N):
            cols = np.empty((I * kh * kw, Hh * Ww), np.float32)
            idx = 0
            for i in range(I):
                for dh in range(kh):
                    for dw in range(kw):
                        cols[idx] = ap[n, i, dh : dh + Hh, dw : dw + Ww].ravel()
                        idx += 1
            out[n] = (wm @ cols).reshape(O, Hh, Ww)
        return out

    def bn(t, g, b, mm, v):
        inv = g / np.sqrt(v + EPS)
        return t * cb(inv) + cb(b - mm * inv)

    def prelu(t, a):
        return np.where(t > 0, t, cb(a) * t)

    s3 = np.mean(np.abs(w3), axis=(1, 2, 3), keepdims=True)
    s1 = np.mean(np.abs(w1), axis=(1, 2, 3), keepdims=True)
    o1 = conv_np(np.sign(x + cb(b11)), np.sign(w3) * s3, 1)
    o1 = x + bn(o1, g1, be1, m1, v1)
    o1 = prelu(o1 + cb(b12), a1) + cb(b13)
    o2 = conv_np(np.sign(o1 + cb(b21)), np.sign(w1) * s1, 0)
    o2 = bn(o2, g2, be2, m2, v2) + o1
    o2 = prelu(o2 + cb(b22), a2) + cb(b23)
    return o2.astype(np.float32)


def kernel(**inputs):
    inputs = {k: np.asarray(v) for k, v in inputs.items()}
    if (np.asarray(inputs["a1"]) > 1).any() or (np.asarray(inputs["a2"]) > 1).any():
        return _fallback_numpy(**{k: np.asarray(v, np.float32)
                                  for k, v in inputs.items()})
    in_maps = make_in_maps(**inputs)
    res = _run(in_maps, trace=False)
    return assemble_out(res.results)
